# revision 20
# baseline (speedup 1.0000x reference)
"""Trainium2 Bass kernel for nn_CrossDimensionalAttention_60550448939365.

Math reduction chain (fast path):

1. scores[b,i,j] = tp[b,i] . fp[b] is constant in j, so softmax over j is
   exactly uniform and attended[b,i,:] = fp[b,:].  Wt/bt/scores/softmax are
   dead code.
2. With c2 = b1 + bo + Wo@b1 == 0 (true for this checkpoint), the second
   residual+projection collapses to y = xn @ W2 with
   W2 = g1[:,None]*(Wo.T + I), xn = LN1core(x + fp[b]).
3. LayerNorm is scale-invariant, so LN2(y) = LN2((z - mean(z)) @ W2) with
   z = x + fp[b]: the entire 1/sigma of LN1 cancels inside LN2.
4. mean-of-z subtraction is a rank-1 correction through W2:
       LN2(z@W2 - mean_h(z) * colsum)        colsum[k] = sum_h W2[h,k]
   and the fp broadcast is folded on the host: x' = x + (fp[b] - mean(fp[b]))
   so that sum_h x'[t,:] is the only per-token scalar needed:
       out = LN2( x'@W2 + sumx[t] * w )      w = -colsum/H
5. out = LN2(...)*g2 + b2 with g2==1, b2==0 (checkpoint) -> plain LN2.

Device work per core (1024 rows): per 128-token tile, 5 accumulating
matmuls into PSUM (1 rank-1 aug with K=1 + 4 contraction chunks, all bf16
operands streamed 512 wide), then bn_stats/bn_aggr + normalize-evacuate
to bf16 and store.  x is uploaded pre-transposed (feature dim on
partitions) in bf16, so there are no on-device transposes and HBM traffic
is halved vs f32.

A general program (the previous-generation kernel) is kept as fallback for
inputs where c2 != 0 or (g2, b2) != (1, 0), so kernel() is correct for any
inputs.

Sharding: rows of flattened [B*S, H] = [8192, 512] split evenly across 8
cores (1024 rows each; each shard lies within one batch b = core//2).
"""

import numpy as np
import ml_dtypes

import concourse.bass as bass
import concourse.tile as tile
from concourse import bacc, mybir
from concourse.bass_utils import run_bass_kernel_spmd
from concourse.masks import make_identity

H = 512
B = 4
S = 2048
N_CORES = 8
ROWS = (B * S) // N_CORES  # 1024 rows per core
P = 128
NT = ROWS // P             # 8 token tiles per core
EPS = 1e-5

F32 = mybir.dt.float32
F32R = mybir.dt.float32r
BF16 = mybir.dt.bfloat16
AF = mybir.ActivationFunctionType
ALU = mybir.AluOpType
NP_BF16 = ml_dtypes.bfloat16


def build_fast_program() -> bass.Bass:
    nc = bacc.Bacc("TRN2", target_bir_lowering=False, debug=False)

    # x layout: row = tile*128 + p(feature-in-chunk), col = chunk*128 + t(token)
    x = nc.dram_tensor("x", [ROWS, H], BF16, kind="ExternalInput").ap()
    # w2 layout: row = p(feature-in-chunk), col = chunk*512 + k
    w2 = nc.dram_tensor("w2", [P, 4 * H], BF16, kind="ExternalInput").ap()
    out = nc.dram_tensor("out", [ROWS, H], BF16, kind="ExternalOutput").ap()

    with tile.TileContext(nc) as tc:
        with (
            tc.tile_pool(name="consts", bufs=1) as consts,
            tc.tile_pool(name="xs", bufs=NT) as xs,
            tc.tile_pool(name="outs", bufs=4) as outs,
            tc.tile_pool(name="stats", bufs=8) as stats,
            tc.tile_pool(name="smalls", bufs=16) as smalls,
            tc.tile_pool(name="psum", bufs=7, space="PSUM") as psum,
            tc.tile_pool(name="psum_w", bufs=1, space="PSUM") as psum_w,
        ):
            epst = consts.tile([P, 1], F32)
            nc.vector.memset(epst, EPS)
            ones1 = consts.tile([P, 1], F32, tag="ones1")
            nc.vector.memset(ones1, 1.0)

            # PE clock warm-up in the DMA shadow: the HAM gate releases the
            # 1.2->2.4 GHz throttle only after a full 4096-cycle window
            # (~3.42us) of gapless PE activity; the first real matmul can't
            # start before the first w2 chunk + x pair land (~11.5us).  Burn
            # the wait on K=128 dummy matmuls over constant zeros so the
            # real stream runs warm from its first op.  The zero-fills go on
            # GpSimd, whose user ops clear the framework preamble ~1us
            # earlier than the vector engine's.
            dl = consts.tile([P, P], BF16, tag="dl")
            nc.gpsimd.memset(dl, 0.0)
            dr = consts.tile([P, H], BF16, tag="dr")
            nc.gpsimd.memset(dr, 0.0)
            pdum = psum_w.tile([P, H], F32)
            for _ in range(6):
                nc.tensor.matmul(pdum, dl, dr, start=True, stop=True)

            # Input DMA runs at the HBM roofline (~300 GB/s shared across
            # queues), so what matters is ARRIVAL ORDER, not queue count:
            # issue everything on the single SP HWDGE ring (strict FIFO) in
            # exact first-use order -- w2 chunk hc arrives right before the
            # first matmul that consumes it, interleaved with the x tiles.
            w2s = consts.tile([P, 4 * H], BF16)
            xts = []
            for hc in range(4):
                nc.sync.dma_start(out=w2s[:, hc * H:(hc + 1) * H],
                                  in_=w2[:, hc * H:(hc + 1) * H])
                xt = xs.tile([P, H], BF16)
                nc.sync.dma_start(out=xt, in_=x[hc * P:(hc + 1) * P, :])
                xts.append(xt)
            for i in range(4, NT):
                xt = xs.tile([P, H], BF16)
                nc.sync.dma_start(out=xt, in_=x[i * P:(i + 1) * P, :])
                xts.append(xt)

            # W2 is row-centered on the host, so the matmul output is
            # already (exactly) mean-free over k: LN2 reduces to
            # y * rsqrt(E[y^2] + eps), no mean subtraction or bias.
            # sqrt/recip are batched per tile PAIR (step-sliced APs) to
            # halve the DVE<->ACT ping-pong count; the final pair keeps
            # per-tile ops with the last evac on DVE so the two tail
            # normalizations run on two engines concurrently.
            for j in range(NT // 2):
                pys, mvp = [], stats.tile([P, 4], F32, tag="mv")
                for a in range(2):
                    i = 2 * j + a
                    xt = xts[i]
                    py = psum.tile([P, H], F32)
                    for hc in range(4):
                        nc.tensor.matmul(
                            py, xt[:, hc * P:(hc + 1) * P],
                            w2s[:, hc * H:(hc + 1) * H],
                            start=(hc == 0), stop=(hc == 3),
                        )
                    pys.append(py)
                    st = stats.tile([P, 6], F32, tag="st")
                    nc.vector.bn_stats(st, py)
                    nc.vector.bn_aggr(mvp[:, 2 * a:2 * a + 2], st)

                last = j == NT // 2 - 1
                if not last:
                    sdp = smalls.tile([P, 2], F32, tag="sd")
                    nc.scalar.activation(sdp, mvp[:, 1:4:2], AF.Sqrt,
                                         bias=epst, scale=1.0)
                    rp = smalls.tile([P, 2], F32, tag="r")
                    nc.vector.reciprocal(rp, sdp)
                    for a in range(2):
                        i = 2 * j + a
                        ot = outs.tile([P, H], BF16)
                        nc.scalar.activation(ot, pys[a], AF.Copy, bias=0.0,
                                             scale=rp[:, a:a + 1])
                        nc.sync.dma_start(out=out[i * P:(i + 1) * P, :],
                                          in_=ot)
                else:
                    for a in range(2):
                        i = 2 * j + a
                        sd = smalls.tile([P, 1], F32, tag="sd")
                        nc.scalar.activation(sd, mvp[:, 2 * a + 1:2 * a + 2],
                                             AF.Sqrt, bias=epst, scale=1.0)
                        r = smalls.tile([P, 1], F32, tag="r")
                        nc.vector.reciprocal(r, sd)
                        ot = outs.tile([P, H], BF16)
                        if a == 1:
                            nc.vector.tensor_scalar(ot, pys[a], r, None,
                                                    op0=ALU.mult)
                        else:
                            nc.scalar.activation(ot, pys[a], AF.Copy,
                                                 bias=0.0, scale=r)
                        nc.sync.dma_start(out=out[i * P:(i + 1) * P, :],
                                          in_=ot)

    nc.compile()
    return nc


def _host_prep_fast(x, static_features, Wf, bf, Wo, g1, b1, bo):
    f32 = np.float32
    fp = static_features @ Wf.T + bf                       # [B,H]
    W2 = g1[:, None] * (Wo.T + np.eye(H, dtype=f32))       # [h,k]
    # LN1's per-token mean subtraction along h is the centering projector
    # C_H = I - 11^T/H on the contraction dim; fold it into the weights.
    # Then center the rows too: x'@W2c with row-centered W2c subtracts
    # exactly mean_k from every output row, so LN2 needs no mean pass.
    W2c = W2 - W2.mean(axis=0, keepdims=True)
    W2c = W2c - W2c.mean(axis=1, keepdims=True)

    xp = (x.reshape(B, S, H) + fp[:, None, :]).reshape(B * S, H)
    xpb = xp.astype(NP_BF16)

    W2b = np.ascontiguousarray(
        W2c.astype(NP_BF16).reshape(4, P, H).transpose(1, 0, 2).reshape(P, 4 * H)
    )

    in_maps = []
    for c in range(N_CORES):
        rows = slice(c * ROWS, (c + 1) * ROWS)
        # [i, t, hc, p] -> [i, p, hc, t]
        xc = xpb[rows].reshape(NT, P, 4, P).transpose(0, 3, 2, 1)
        in_maps.append({
            "x": np.ascontiguousarray(xc).reshape(ROWS, H),
            "w2": W2b,
        })
    return in_maps


# ---------------------------------------------------------------------------
# General fallback path (previous-generation kernel): correct for any c2,
# g2, b2.  Only used when the checkpoint does not satisfy the fast-path
# preconditions, so its performance does not matter.
# ---------------------------------------------------------------------------

def _bcast_ap(src: bass.AP, parts: int) -> bass.AP:
    return bass.AP(tensor=src.tensor, offset=src.offset, ap=[[0, parts]] + list(src.ap))


def _row_ap(src: bass.AP) -> bass.AP:
    return bass.AP(tensor=src.tensor, offset=src.offset, ap=[[0, 1]] + list(src.ap))


def build_general_program(with_c2: bool, with_affine2: bool) -> bass.Bass:
    nc = bacc.Bacc("TRN2", target_bir_lowering=False, debug=False)

    x = nc.dram_tensor("x", [ROWS, H], F32, kind="ExternalInput").ap()
    w2 = nc.dram_tensor("w2", [H, H], F32, kind="ExternalInput").ap()
    c2 = nc.dram_tensor("c2", [H], F32, kind="ExternalInput").ap()
    fp = nc.dram_tensor("fp", [H], F32, kind="ExternalInput").ap()
    g2 = nc.dram_tensor("g2", [H], F32, kind="ExternalInput").ap()
    b2 = nc.dram_tensor("b2", [H], F32, kind="ExternalInput").ap()
    out = nc.dram_tensor("out", [ROWS, H], F32, kind="ExternalOutput").ap()

    MD = F32R

    with tile.TileContext(nc) as tc:
        with (
            tc.tile_pool(name="consts", bufs=1) as consts,
            tc.tile_pool(name="xs", bufs=4) as xs,
            tc.tile_pool(name="zs", bufs=4) as zs,
            tc.tile_pool(name="xns", bufs=8) as xns,
            tc.tile_pool(name="xnts", bufs=3) as xnts,
            tc.tile_pool(name="stats", bufs=6) as stats,
            tc.tile_pool(name="smalls", bufs=12) as smalls,
            tc.tile_pool(name="ts", bufs=3) as ts_pool,
            tc.tile_pool(name="outs", bufs=3) as outs,
            tc.tile_pool(name="psum_t", bufs=3, space="PSUM") as psum_t,
            tc.tile_pool(name="psum_y", bufs=3, space="PSUM") as psum_y,
            tc.tile_pool(name="psum_d", bufs=1, space="PSUM") as psum_d,
        ):
            ones1 = consts.tile([1, P], F32)
            nc.vector.memset(ones1, 1.0)
            onesmm = consts.tile([1, P], MD)
            nc.vector.tensor_copy(onesmm, ones1)

            fprow = consts.tile([1, H], F32)
            nc.sync.dma_start(out=fprow, in_=_row_ap(fp))
            fpmm = consts.tile([1, H], MD)
            nc.vector.tensor_copy(fpmm, fprow)
            fp_ps = psum_d.tile([P, H], F32, tag="bcast")
            nc.tensor.matmul(fp_ps, onesmm, fpmm, start=True, stop=True)
            fpb = consts.tile([P, H], F32)
            nc.scalar.copy(fpb, fp_ps)

            if with_affine2:
                g2b = consts.tile([P, H], F32)
                nc.gpsimd.dma_start(out=g2b, in_=_bcast_ap(g2, P))
                b2b = consts.tile([P, H], F32)
                nc.gpsimd.dma_start(out=b2b, in_=_bcast_ap(b2, P))

            if with_c2:
                c2row = consts.tile([1, H], F32)
                nc.sync.dma_start(out=c2row, in_=_row_ap(c2))
                c2mm = consts.tile([1, H], MD)
                nc.vector.tensor_copy(c2mm, c2row)

            iden_f32 = consts.tile([P, P], F32)
            make_identity(nc, iden_f32)
            iden = consts.tile([P, P], F32R)
            nc.gpsimd.tensor_copy(iden, iden_f32)
            epst = consts.tile([P, 1], F32)
            nc.vector.memset(epst, EPS)

            d1 = psum_d.tile([P, P], MD, tag="dummy")
            nc.tensor.transpose(d1, iden, iden)

            xn_all, xnt_all = {}, {}
            w2mm = consts.tile([P, 4, H], MD)
            for i in range(NT + 3):
                if i == 1:
                    w2s = consts.tile([P, 4, H], F32)
                    nc.sync.dma_start(
                        out=w2s, in_=w2.rearrange("(t p) k -> p t k", p=P)
                    )
                    nc.scalar.copy(w2mm, w2s)

                if i < NT:
                    xt = xs.tile([P, H], F32)
                    nc.sync.dma_start(out=xt, in_=x[i * P:(i + 1) * P, :])

                    z = zs.tile([P, H], F32)
                    nc.vector.tensor_add(z, xt, fpb)

                    st1 = stats.tile([P, 6], F32, tag="st")
                    nc.vector.bn_stats(st1, z)
                    mv1 = stats.tile([P, 2], F32, tag="mv")
                    nc.vector.bn_aggr(mv1, st1)
                    sd1 = smalls.tile([P, 1], F32, tag="sd")
                    nc.scalar.activation(sd1, mv1[:, 1:2], AF.Sqrt, bias=epst,
                                         scale=1.0)
                    s1 = smalls.tile([P, 1], F32, tag="s")
                    nc.vector.reciprocal(s1, sd1)
                    negms1 = smalls.tile([P, 1], F32, tag="negms")
                    nc.vector.tensor_scalar(
                        negms1, mv1[:, 0:1], s1, -1.0, op0=ALU.mult, op1=ALU.mult
                    )
                    xn = xns.tile([P, H], MD)
                    nc.scalar.activation(xn, z, AF.Identity, bias=negms1, scale=s1)
                    xn_all[i] = xn

                if 2 <= i < NT + 2:
                    j = i - 2
                    xn = xn_all[j]
                    ptr = psum_t.tile([P, 4, P], MD)
                    for h in range(4):
                        nc.tensor.transpose(ptr[:, h, :], xn[:, h * P:(h + 1) * P],
                                            iden)
                    xnt = xnts.tile([P, 4, P], MD)
                    nc.scalar.copy(xnt, ptr)
                    xnt_all[j] = xnt

                if i >= 3:
                    k = i - 3
                    xnt = xnt_all[k]
                    py = psum_y.tile([P, H], F32)
                    if with_c2:
                        nc.tensor.matmul(py, onesmm, c2mm, start=True, stop=False)
                    for h in range(4):
                        nc.tensor.matmul(
                            py, xnt[:, h, :], w2mm[:, h, :],
                            start=(h == 0 and not with_c2), stop=(h == 3),
                        )

                    st2 = stats.tile([P, 6], F32, tag="st")
                    nc.vector.bn_stats(st2, py)
                    mv2 = stats.tile([P, 2], F32, tag="mv")
                    nc.vector.bn_aggr(mv2, st2)
                    sd2 = smalls.tile([P, 1], F32, tag="sd")
                    nc.scalar.activation(sd2, mv2[:, 1:2], AF.Sqrt, bias=epst,
                                         scale=1.0)
                    s2 = smalls.tile([P, 1], F32, tag="s")
                    nc.vector.reciprocal(s2, sd2)
                    negms2 = smalls.tile([P, 1], F32, tag="negms")
                    nc.vector.tensor_scalar(
                        negms2, mv2[:, 0:1], s2, -1.0, op0=ALU.mult, op1=ALU.mult
                    )

                    t = ts_pool.tile([P, H], F32)
                    nc.scalar.activation(t, py, AF.Identity, bias=negms2, scale=s2)

                    if with_affine2:
                        t2 = outs.tile([P, H], F32, tag="t2")
                        nc.gpsimd.tensor_mul(t2, t, g2b)
                        ot = outs.tile([P, H], F32, tag="ot")
                        nc.gpsimd.tensor_add(ot, t2, b2b)
                    else:
                        ot = t

                    nc.sync.dma_start(out=out[k * P:(k + 1) * P, :], in_=ot)

    nc.compile()
    return nc


def _host_prep_general(x, static_features, Wf, bf, Wo, bo, g1, b1, g2, b2):
    f32 = np.float32
    fp = static_features @ Wf.T + bf
    W2 = g1[:, None] * (Wo.T + np.eye(H, dtype=f32))
    c2 = b1 + bo + Wo @ b1

    in_maps = []
    for c in range(N_CORES):
        shard = np.ascontiguousarray(x[c * ROWS:(c + 1) * ROWS])
        in_maps.append({
            "x": shard,
            "w2": np.ascontiguousarray(W2),
            "c2": np.ascontiguousarray(c2),
            "fp": np.ascontiguousarray(fp[(c * ROWS) // S]),
            "g2": np.ascontiguousarray(g2),
            "b2": np.ascontiguousarray(b2),
        })
    return in_maps


_NC_CACHE = {}


def _get_program(key, builder, *args):
    if key not in _NC_CACHE:
        _NC_CACHE[key] = builder(*args)
    return _NC_CACHE[key]


def run(inputs: dict, trace: bool = False):
    """Returns (output [B,S,H] f32, BassKernelResults)."""
    f32 = np.float32
    x = np.ascontiguousarray(
        np.asarray(inputs["temporal_features"], dtype=f32)
    ).reshape(B * S, H)
    st = np.asarray(inputs["static_features"], dtype=f32)
    Wf = np.asarray(inputs["Wf"], dtype=f32)
    bf = np.asarray(inputs["bf"], dtype=f32)
    Wo = np.asarray(inputs["Wo"], dtype=f32)
    bo = np.asarray(inputs["bo"], dtype=f32)
    g1 = np.asarray(inputs["g1"], dtype=f32)
    b1 = np.asarray(inputs["b1"], dtype=f32)
    g2 = np.asarray(inputs["g2"], dtype=f32)
    b2 = np.asarray(inputs["b2"], dtype=f32)

    c2 = b1 + bo + Wo @ b1
    fast = (
        not np.any(c2 != 0.0)
        and not np.any(g2 != 1.0)
        and not np.any(b2 != 0.0)
    )

    if fast:
        in_maps = _host_prep_fast(x, st, Wf, bf, Wo, g1, b1, bo)
        nc = _get_program("fast", build_fast_program)
        res = run_bass_kernel_spmd(nc, in_maps, list(range(N_CORES)), trace=trace)
        shards = [res.results[c]["out"] for c in range(N_CORES)]
        full = np.concatenate(shards, axis=0).astype(f32).reshape(B, S, H)
    else:
        in_maps = _host_prep_general(x, st, Wf, bf, Wo, bo, g1, b1, g2, b2)
        with_c2 = bool(np.any(c2 != 0.0))
        with_affine2 = bool(np.any(g2 != 1.0) or np.any(b2 != 0.0))
        nc = _get_program(("gen", with_c2, with_affine2), build_general_program,
                          with_c2, with_affine2)
        res = run_bass_kernel_spmd(nc, in_maps, list(range(N_CORES)), trace=trace)
        shards = [res.results[c]["out"] for c in range(N_CORES)]
        full = np.concatenate(shards, axis=0).reshape(B, S, H).astype(f32)
    return full, res


def kernel(**inputs) -> np.ndarray:
    out, _ = run(inputs, trace=False)
    return out


# revision 24
# speedup vs baseline: 1.0298x; 1.0298x over previous
"""Trainium2 Bass kernel for nn_CrossDimensionalAttention_60550448939365.

Math reduction chain (fast path):

1. scores[b,i,j] = tp[b,i] . fp[b] is constant in j, so softmax over j is
   exactly uniform and attended[b,i,:] = fp[b,:].  Wt/bt/scores/softmax are
   dead code.
2. With c2 = b1 + bo + Wo@b1 == 0 (true for this checkpoint), the second
   residual+projection collapses to y = xn @ W2 with
   W2 = g1[:,None]*(Wo.T + I), xn = LN1core(x + fp[b]).
3. LayerNorm is scale-invariant, so LN2(y) = LN2((z - mean(z)) @ W2) with
   z = x + fp[b]: the entire 1/sigma of LN1 cancels inside LN2.
4. mean-of-z subtraction is a rank-1 correction through W2:
       LN2(z@W2 - mean_h(z) * colsum)        colsum[k] = sum_h W2[h,k]
   and the fp broadcast is folded on the host: x' = x + (fp[b] - mean(fp[b]))
   so that sum_h x'[t,:] is the only per-token scalar needed:
       out = LN2( x'@W2 + sumx[t] * w )      w = -colsum/H
5. out = LN2(...)*g2 + b2 with g2==1, b2==0 (checkpoint) -> plain LN2.

Device work per core (1024 rows): per 128-token tile, 5 accumulating
matmuls into PSUM (1 rank-1 aug with K=1 + 4 contraction chunks, all bf16
operands streamed 512 wide), then bn_stats/bn_aggr + normalize-evacuate
to bf16 and store.  x is uploaded pre-transposed (feature dim on
partitions) in bf16, so there are no on-device transposes and HBM traffic
is halved vs f32.

A general program (the previous-generation kernel) is kept as fallback for
inputs where c2 != 0 or (g2, b2) != (1, 0), so kernel() is correct for any
inputs.

Sharding: rows of flattened [B*S, H] = [8192, 512] split evenly across 8
cores (1024 rows each; each shard lies within one batch b = core//2).
"""

import numpy as np
import ml_dtypes

import concourse.bass as bass
import concourse.tile as tile
from concourse import bacc, mybir
from concourse.bass_utils import run_bass_kernel_spmd
from concourse.masks import make_identity

H = 512
B = 4
S = 2048
N_CORES = 8
ROWS = (B * S) // N_CORES  # 1024 rows per core
P = 128
NT = ROWS // P             # 8 token tiles per core
EPS = 1e-5

F32 = mybir.dt.float32
F32R = mybir.dt.float32r
BF16 = mybir.dt.bfloat16
AF = mybir.ActivationFunctionType
ALU = mybir.AluOpType
NP_BF16 = ml_dtypes.bfloat16


def build_fast_program() -> bass.Bass:
    nc = bacc.Bacc("TRN2", target_bir_lowering=False, debug=False)

    # x layout: row = tile*128 + p(feature-in-chunk), col = chunk*128 + t(token)
    x = nc.dram_tensor("x", [ROWS, H], BF16, kind="ExternalInput").ap()
    # w2 layout: row = p(feature-in-chunk), col = chunk*512 + k
    w2 = nc.dram_tensor("w2", [P, 4 * H], BF16, kind="ExternalInput").ap()
    out = nc.dram_tensor("out", [ROWS, H], BF16, kind="ExternalOutput").ap()

    with tile.TileContext(nc) as tc:
        with (
            tc.tile_pool(name="consts", bufs=1) as consts,
            tc.tile_pool(name="xs", bufs=NT) as xs,
            tc.tile_pool(name="outs", bufs=4) as outs,
            tc.tile_pool(name="stats", bufs=8) as stats,
            tc.tile_pool(name="smalls", bufs=16) as smalls,
            tc.tile_pool(name="psum", bufs=7, space="PSUM") as psum,
            tc.tile_pool(name="psum_w", bufs=1, space="PSUM") as psum_w,
        ):
            epst = consts.tile([P, 1], F32)
            nc.vector.memset(epst, EPS)
            ones1 = consts.tile([P, 1], F32, tag="ones1")
            nc.vector.memset(ones1, 1.0)

            # PE clock warm-up in the DMA shadow: the HAM gate releases the
            # 1.2->2.4 GHz throttle only after a full 4096-cycle window
            # (~3.42us) of gapless PE activity; the first real matmul can't
            # start before the first w2 chunk + x pair land (~11.5us).  Burn
            # the wait on K=128 dummy matmuls over constant zeros so the
            # real stream runs warm from its first op.  The zero-fills go on
            # GpSimd, whose user ops clear the framework preamble ~1us
            # earlier than the vector engine's.
            dl = consts.tile([P, P], BF16, tag="dl")
            nc.gpsimd.memset(dl, 0.0)
            dr = consts.tile([P, H], BF16, tag="dr")
            nc.gpsimd.memset(dr, 0.0)
            pdum = psum_w.tile([P, H], F32)
            for _ in range(6):
                nc.tensor.matmul(pdum, dl, dr, start=True, stop=True)

            # Input DMA runs at the HBM roofline (~300 GB/s shared across
            # queues), so what matters is ARRIVAL ORDER, not queue count:
            # issue everything on the single SP HWDGE ring (strict FIFO) in
            # exact first-use order -- w2 chunk hc arrives right before the
            # first matmul that consumes it, interleaved with the x tiles.
            w2s = consts.tile([P, 4 * H], BF16)
            xts = []
            for hc in range(4):
                nc.sync.dma_start(out=w2s[:, hc * H:(hc + 1) * H],
                                  in_=w2[:, hc * H:(hc + 1) * H])
                xt = xs.tile([P, H], BF16)
                nc.sync.dma_start(out=xt, in_=x[hc * P:(hc + 1) * P, :])
                xts.append(xt)
            for i in range(4, NT):
                xt = xs.tile([P, H], BF16)
                nc.sync.dma_start(out=xt, in_=x[i * P:(i + 1) * P, :])
                xts.append(xt)

            # W2 is row-centered on the host, so the matmul output is
            # already (exactly) mean-free over k: LN2 reduces to
            # y * rsqrt(E[y^2] + eps), no mean subtraction or bias.
            # sqrt/recip are batched per tile PAIR (step-sliced APs) to
            # halve the DVE<->ACT ping-pong count; the final pair keeps
            # per-tile ops with the last evac on DVE so the two tail
            # normalizations run on two engines concurrently.
            for j in range(NT // 2):
                pys, mvp = [], stats.tile([P, 4], F32, tag="mv")
                for a in range(2):
                    i = 2 * j + a
                    xt = xts[i]
                    py = psum.tile([P, H], F32)
                    for hc in range(4):
                        nc.tensor.matmul(
                            py, xt[:, hc * P:(hc + 1) * P],
                            w2s[:, hc * H:(hc + 1) * H],
                            start=(hc == 0), stop=(hc == 3),
                        )
                    pys.append(py)
                    st = stats.tile([P, 6], F32, tag="st")
                    nc.vector.bn_stats(st, py)
                    nc.vector.bn_aggr(mvp[:, 2 * a:2 * a + 2], st)

                last = j == NT // 2 - 1
                if not last:
                    sdp = smalls.tile([P, 2], F32, tag="sd")
                    nc.scalar.activation(sdp, mvp[:, 1:4:2], AF.Sqrt,
                                         bias=epst, scale=1.0)
                    rp = smalls.tile([P, 2], F32, tag="r")
                    nc.vector.reciprocal(rp, sdp)
                    for a in range(2):
                        i = 2 * j + a
                        ot = outs.tile([P, H], BF16)
                        nc.scalar.activation(ot, pys[a], AF.Copy, bias=0.0,
                                             scale=rp[:, a:a + 1])
                        nc.sync.dma_start(out=out[i * P:(i + 1) * P, :],
                                          in_=ot)
                else:
                    for a in range(2):
                        i = 2 * j + a
                        sd = smalls.tile([P, 1], F32, tag="sd")
                        nc.scalar.activation(sd, mvp[:, 2 * a + 1:2 * a + 2],
                                             AF.Sqrt, bias=epst, scale=1.0)
                        r = smalls.tile([P, 1], F32, tag="r")
                        nc.vector.reciprocal(r, sd)
                        ot = outs.tile([P, H], BF16)
                        if a == 1:
                            nc.vector.tensor_scalar(ot, pys[a], r, None,
                                                    op0=ALU.mult)
                        else:
                            nc.scalar.activation(ot, pys[a], AF.Copy,
                                                 bias=0.0, scale=r)
                        nc.sync.dma_start(out=out[i * P:(i + 1) * P, :],
                                          in_=ot)

    nc.compile()
    return nc


def _host_prep_fast(x, static_features, Wf, bf, Wo, g1, b1, bo):
    f32 = np.float32
    fp = static_features @ Wf.T + bf                       # [B,H]
    W2 = g1[:, None] * (Wo.T + np.eye(H, dtype=f32))       # [h,k]
    # LN1's per-token mean subtraction along h is the centering projector
    # C_H = I - 11^T/H on the contraction dim; fold it into the weights.
    # Then center the rows too: x'@W2c with row-centered W2c subtracts
    # exactly mean_k from every output row, so LN2 needs no mean pass.
    W2c = W2 - W2.mean(axis=0, keepdims=True)
    W2c = W2c - W2c.mean(axis=1, keepdims=True)

    xp = (x.reshape(B, S, H) + fp[:, None, :]).reshape(B * S, H)
    xpb = xp.astype(NP_BF16)

    W2b = np.ascontiguousarray(
        W2c.astype(NP_BF16).reshape(4, P, H).transpose(1, 0, 2).reshape(P, 4 * H)
    )

    in_maps = []
    for c in range(N_CORES):
        rows = slice(c * ROWS, (c + 1) * ROWS)
        # [i, t, hc, p] -> [i, p, hc, t]
        xc = xpb[rows].reshape(NT, P, 4, P).transpose(0, 3, 2, 1)
        in_maps.append({
            "x": np.ascontiguousarray(xc).reshape(ROWS, H),
            "w2": W2b,
        })
    return in_maps


# ---------------------------------------------------------------------------
# General fallback path (previous-generation kernel): correct for any c2,
# g2, b2.  Only used when the checkpoint does not satisfy the fast-path
# preconditions, so its performance does not matter.
# ---------------------------------------------------------------------------

def _bcast_ap(src: bass.AP, parts: int) -> bass.AP:
    return bass.AP(tensor=src.tensor, offset=src.offset, ap=[[0, parts]] + list(src.ap))


def _row_ap(src: bass.AP) -> bass.AP:
    return bass.AP(tensor=src.tensor, offset=src.offset, ap=[[0, 1]] + list(src.ap))


def build_general_program(with_c2: bool, with_affine2: bool) -> bass.Bass:
    nc = bacc.Bacc("TRN2", target_bir_lowering=False, debug=False)

    x = nc.dram_tensor("x", [ROWS, H], F32, kind="ExternalInput").ap()
    w2 = nc.dram_tensor("w2", [H, H], F32, kind="ExternalInput").ap()
    c2 = nc.dram_tensor("c2", [H], F32, kind="ExternalInput").ap()
    fp = nc.dram_tensor("fp", [H], F32, kind="ExternalInput").ap()
    g2 = nc.dram_tensor("g2", [H], F32, kind="ExternalInput").ap()
    b2 = nc.dram_tensor("b2", [H], F32, kind="ExternalInput").ap()
    out = nc.dram_tensor("out", [ROWS, H], F32, kind="ExternalOutput").ap()

    MD = F32R

    with tile.TileContext(nc) as tc:
        with (
            tc.tile_pool(name="consts", bufs=1) as consts,
            tc.tile_pool(name="xs", bufs=4) as xs,
            tc.tile_pool(name="zs", bufs=4) as zs,
            tc.tile_pool(name="xns", bufs=8) as xns,
            tc.tile_pool(name="xnts", bufs=3) as xnts,
            tc.tile_pool(name="stats", bufs=6) as stats,
            tc.tile_pool(name="smalls", bufs=12) as smalls,
            tc.tile_pool(name="ts", bufs=3) as ts_pool,
            tc.tile_pool(name="outs", bufs=3) as outs,
            tc.tile_pool(name="psum_t", bufs=3, space="PSUM") as psum_t,
            tc.tile_pool(name="psum_y", bufs=3, space="PSUM") as psum_y,
            tc.tile_pool(name="psum_d", bufs=1, space="PSUM") as psum_d,
        ):
            ones1 = consts.tile([1, P], F32)
            nc.vector.memset(ones1, 1.0)
            onesmm = consts.tile([1, P], MD)
            nc.vector.tensor_copy(onesmm, ones1)

            fprow = consts.tile([1, H], F32)
            nc.sync.dma_start(out=fprow, in_=_row_ap(fp))
            fpmm = consts.tile([1, H], MD)
            nc.vector.tensor_copy(fpmm, fprow)
            fp_ps = psum_d.tile([P, H], F32, tag="bcast")
            nc.tensor.matmul(fp_ps, onesmm, fpmm, start=True, stop=True)
            fpb = consts.tile([P, H], F32)
            nc.scalar.copy(fpb, fp_ps)

            if with_affine2:
                g2b = consts.tile([P, H], F32)
                nc.gpsimd.dma_start(out=g2b, in_=_bcast_ap(g2, P))
                b2b = consts.tile([P, H], F32)
                nc.gpsimd.dma_start(out=b2b, in_=_bcast_ap(b2, P))

            if with_c2:
                c2row = consts.tile([1, H], F32)
                nc.sync.dma_start(out=c2row, in_=_row_ap(c2))
                c2mm = consts.tile([1, H], MD)
                nc.vector.tensor_copy(c2mm, c2row)

            iden_f32 = consts.tile([P, P], F32)
            make_identity(nc, iden_f32)
            iden = consts.tile([P, P], F32R)
            nc.gpsimd.tensor_copy(iden, iden_f32)
            epst = consts.tile([P, 1], F32)
            nc.vector.memset(epst, EPS)

            d1 = psum_d.tile([P, P], MD, tag="dummy")
            nc.tensor.transpose(d1, iden, iden)

            xn_all, xnt_all = {}, {}
            w2mm = consts.tile([P, 4, H], MD)
            for i in range(NT + 3):
                if i == 1:
                    w2s = consts.tile([P, 4, H], F32)
                    nc.sync.dma_start(
                        out=w2s, in_=w2.rearrange("(t p) k -> p t k", p=P)
                    )
                    nc.scalar.copy(w2mm, w2s)

                if i < NT:
                    xt = xs.tile([P, H], F32)
                    nc.sync.dma_start(out=xt, in_=x[i * P:(i + 1) * P, :])

                    z = zs.tile([P, H], F32)
                    nc.vector.tensor_add(z, xt, fpb)

                    st1 = stats.tile([P, 6], F32, tag="st")
                    nc.vector.bn_stats(st1, z)
                    mv1 = stats.tile([P, 2], F32, tag="mv")
                    nc.vector.bn_aggr(mv1, st1)
                    sd1 = smalls.tile([P, 1], F32, tag="sd")
                    nc.scalar.activation(sd1, mv1[:, 1:2], AF.Sqrt, bias=epst,
                                         scale=1.0)
                    s1 = smalls.tile([P, 1], F32, tag="s")
                    nc.vector.reciprocal(s1, sd1)
                    negms1 = smalls.tile([P, 1], F32, tag="negms")
                    nc.vector.tensor_scalar(
                        negms1, mv1[:, 0:1], s1, -1.0, op0=ALU.mult, op1=ALU.mult
                    )
                    xn = xns.tile([P, H], MD)
                    nc.scalar.activation(xn, z, AF.Identity, bias=negms1, scale=s1)
                    xn_all[i] = xn

                if 2 <= i < NT + 2:
                    j = i - 2
                    xn = xn_all[j]
                    ptr = psum_t.tile([P, 4, P], MD)
                    for h in range(4):
                        nc.tensor.transpose(ptr[:, h, :], xn[:, h * P:(h + 1) * P],
                                            iden)
                    xnt = xnts.tile([P, 4, P], MD)
                    nc.scalar.copy(xnt, ptr)
                    xnt_all[j] = xnt

                if i >= 3:
                    k = i - 3
                    xnt = xnt_all[k]
                    py = psum_y.tile([P, H], F32)
                    if with_c2:
                        nc.tensor.matmul(py, onesmm, c2mm, start=True, stop=False)
                    for h in range(4):
                        nc.tensor.matmul(
                            py, xnt[:, h, :], w2mm[:, h, :],
                            start=(h == 0 and not with_c2), stop=(h == 3),
                        )

                    st2 = stats.tile([P, 6], F32, tag="st")
                    nc.vector.bn_stats(st2, py)
                    mv2 = stats.tile([P, 2], F32, tag="mv")
                    nc.vector.bn_aggr(mv2, st2)
                    sd2 = smalls.tile([P, 1], F32, tag="sd")
                    nc.scalar.activation(sd2, mv2[:, 1:2], AF.Sqrt, bias=epst,
                                         scale=1.0)
                    s2 = smalls.tile([P, 1], F32, tag="s")
                    nc.vector.reciprocal(s2, sd2)
                    negms2 = smalls.tile([P, 1], F32, tag="negms")
                    nc.vector.tensor_scalar(
                        negms2, mv2[:, 0:1], s2, -1.0, op0=ALU.mult, op1=ALU.mult
                    )

                    t = ts_pool.tile([P, H], F32)
                    nc.scalar.activation(t, py, AF.Identity, bias=negms2, scale=s2)

                    if with_affine2:
                        t2 = outs.tile([P, H], F32, tag="t2")
                        nc.gpsimd.tensor_mul(t2, t, g2b)
                        ot = outs.tile([P, H], F32, tag="ot")
                        nc.gpsimd.tensor_add(ot, t2, b2b)
                    else:
                        ot = t

                    nc.sync.dma_start(out=out[k * P:(k + 1) * P, :], in_=ot)

    nc.compile()
    return nc


def _host_prep_general(x, static_features, Wf, bf, Wo, bo, g1, b1, g2, b2):
    f32 = np.float32
    fp = static_features @ Wf.T + bf
    W2 = g1[:, None] * (Wo.T + np.eye(H, dtype=f32))
    c2 = b1 + bo + Wo @ b1

    in_maps = []
    for c in range(N_CORES):
        shard = np.ascontiguousarray(x[c * ROWS:(c + 1) * ROWS])
        in_maps.append({
            "x": shard,
            "w2": np.ascontiguousarray(W2),
            "c2": np.ascontiguousarray(c2),
            "fp": np.ascontiguousarray(fp[(c * ROWS) // S]),
            "g2": np.ascontiguousarray(g2),
            "b2": np.ascontiguousarray(b2),
        })
    return in_maps


_NC_CACHE = {}


def _get_program(key, builder, *args):
    if key not in _NC_CACHE:
        _NC_CACHE[key] = builder(*args)
    return _NC_CACHE[key]


def run(inputs: dict, trace: bool = False):
    """Returns (output [B,S,H] f32, BassKernelResults)."""
    f32 = np.float32
    x = np.ascontiguousarray(
        np.asarray(inputs["temporal_features"], dtype=f32)
    ).reshape(B * S, H)
    st = np.asarray(inputs["static_features"], dtype=f32)
    Wf = np.asarray(inputs["Wf"], dtype=f32)
    bf = np.asarray(inputs["bf"], dtype=f32)
    Wo = np.asarray(inputs["Wo"], dtype=f32)
    bo = np.asarray(inputs["bo"], dtype=f32)
    g1 = np.asarray(inputs["g1"], dtype=f32)
    b1 = np.asarray(inputs["b1"], dtype=f32)
    g2 = np.asarray(inputs["g2"], dtype=f32)
    b2 = np.asarray(inputs["b2"], dtype=f32)

    c2 = b1 + bo + Wo @ b1
    fast = (
        not np.any(c2 != 0.0)
        and not np.any(g2 != 1.0)
        and not np.any(b2 != 0.0)
    )

    if fast:
        in_maps = _host_prep_fast(x, st, Wf, bf, Wo, g1, b1, bo)
        nc = _get_program("fast", build_fast_program)
        res = run_bass_kernel_spmd(nc, in_maps, list(range(N_CORES)), trace=trace)
        shards = [res.results[c]["out"] for c in range(N_CORES)]
        full = np.concatenate(shards, axis=0).astype(f32).reshape(B, S, H)
    else:
        in_maps = _host_prep_general(x, st, Wf, bf, Wo, bo, g1, b1, g2, b2)
        with_c2 = bool(np.any(c2 != 0.0))
        with_affine2 = bool(np.any(g2 != 1.0) or np.any(b2 != 0.0))
        nc = _get_program(("gen", with_c2, with_affine2), build_general_program,
                          with_c2, with_affine2)
        res = run_bass_kernel_spmd(nc, in_maps, list(range(N_CORES)), trace=trace)
        shards = [res.results[c]["out"] for c in range(N_CORES)]
        full = np.concatenate(shards, axis=0).reshape(B, S, H).astype(f32)
    return full, res


def kernel(**inputs) -> np.ndarray:
    out, _ = run(inputs, trace=False)
    return out
